# revision 6
# baseline (speedup 1.0000x reference)
"""Trainium2 Bass kernel for AttentionL2 (B=4, S=4096, DIN=384, DOUT=64).

out = softmax(cdist(q, k) / 8, axis=-1) @ v  with q/k/v = x @ W{q,k,v}.T

Sharding: 8 cores = 4 batches x 2 query-halves. Each core receives the
full x of its batch with rows reordered so its own query half comes
first (softmax over keys is permutation invariant, so reordering keys is
harmless). Every core is then the *same* SPMD program:
  - q rows = x rows 0:2048, keys = all 4096 rows.

Math on core (all matmuls bf16 with fp32 accumulation):
  d2[j,i] = |q_i - k_j|^2 via one augmented matmul:
      lhsT = [-2*k^T; k2; ones]  (66 x 128 keys per tile)
      rhs  = [q^T; ones; q2]     (66 x 2048)
  dist/8 = Sqrt(d2/64)       (ScalarE, phase 1, into fp16 SBUF buffer)
  att    = Exp(dist/8)       (ScalarE, phase 2; unnormalized - distances
                              are O(10) so exp never overflows, and
                              softmax needs no max subtraction)
  outT   = [v, 1]^T @ att    (row 64 = softmax denominator)
  out    = (outT[0:64] / outT[64]).T
The two activation functions live in different ACT table sets, so the
phases are separated by a scheduler barrier: all Sqrt, one table switch,
all Exp.
"""

from contextlib import ExitStack

import numpy as np

import concourse.bacc as bacc
import concourse.mybir as mybir
import concourse.tile as tile
from concourse import masks
from concourse.bass_utils import run_bass_kernel_spmd

F32 = mybir.dt.float32
BF16 = mybir.dt.bfloat16
F16 = mybir.dt.float16
AF = mybir.ActivationFunctionType

B, S, DIN, DOUT = 4, 4096, 384, 64
M = S // 2        # query rows per core
KT = S // 128     # 32 key tiles
QS = M // 512     # 4 query chunks of 512
DC = DIN // 128   # 3 contraction chunks
NCORES = 8


def _body(tc, x, wq, wk, wv, out):
    nc = tc.nc
    with ExitStack() as ctx:
        const_pool = ctx.enter_context(tc.tile_pool(name="const", bufs=1))
        ident = const_pool.tile([128, 128], F32)
        masks.make_identity(nc, ident[:])
        ones64 = const_pool.tile([64, 1], BF16)
        nc.vector.memset(ones64[:], 1.0)
        ones64x2 = const_pool.tile([64, 2], BF16)
        nc.vector.memset(ones64x2[:], 1.0)

        main_pool = ctx.enter_context(tc.tile_pool(name="main", bufs=1))
        kT_aug = main_pool.tile([66, S], BF16)
        qT_aug = main_pool.tile([66, M], BF16)
        v_sb = main_pool.tile([128, KT, 65], BF16)

        # ---------------- setup: load, transpose, project ----------------
        with ExitStack() as sctx:
            xp = sctx.enter_context(tc.tile_pool(name="xsb", bufs=1))
            x_sb = xp.tile([128, KT, DIN], F32)
            x_r = x.rearrange("(n p) d -> p n d", p=128)
            for g in range(8):
                nc.sync.dma_start(
                    x_sb[:, 4 * g : 4 * (g + 1), :], x_r[:, 4 * g : 4 * (g + 1), :]
                )
            w_sb = xp.tile([64, 3, DIN], F32)
            for w_i, w in enumerate((wq, wk, wv)):
                nc.sync.dma_start(w_sb[:, w_i, :], w)

            # W^T chunks, bf16: wT[:, w_i, c, :] = W[w_i][:, 128c:128c+128].T
            wT = xp.tile([128, 3, DC, 64], BF16)
            pw_pool = sctx.enter_context(tc.tile_pool(name="pw", bufs=2, space="PSUM"))
            for w_i in range(3):
                for c in range(DC):
                    pw = pw_pool.tile([128, 64], F32)
                    nc.tensor.transpose(
                        pw[:], w_sb[:, w_i, c * 128 : (c + 1) * 128], ident[0:64, 0:64]
                    )
                    nc.any.tensor_copy(wT[:, w_i, c, :], pw[:])

            # x^T chunks, bf16: xT[:, c, s] = x[s, 128c + p]
            xT = xp.tile([128, DC, S], BF16)
            pxt_pool = sctx.enter_context(
                tc.tile_pool(name="pxt", bufs=3, space="PSUM")
            )
            for c in range(DC):
                for n4 in range(KT // 4):
                    pxt = pxt_pool.tile([128, 512], F32)
                    for j in range(4):
                        n = n4 * 4 + j
                        nc.tensor.transpose(
                            pxt[:, j * 128 : (j + 1) * 128],
                            x_sb[:, n, c * 128 : (c + 1) * 128],
                            ident[:],
                        )
                    nc.any.tensor_copy(xT[:, c, n4 * 512 : (n4 + 1) * 512], pxt[:])

            pp_pool = sctx.enter_context(tc.tile_pool(name="pp", bufs=3, space="PSUM"))
            # k rows: kT_aug[0:64] = -2 * k^T
            for ss in range(S // 512):
                pk = pp_pool.tile([64, 512], F32, tag="p")
                for c in range(DC):
                    nc.tensor.matmul(
                        pk[:],
                        wT[:, 1, c, :],
                        xT[:, c, ss * 512 : (ss + 1) * 512],
                        start=(c == 0),
                        stop=(c == DC - 1),
                    )
                nc.vector.tensor_scalar_mul(
                    kT_aug[0:64, ss * 512 : (ss + 1) * 512], pk[:], -2.0
                )
            # q rows: qT_aug[0:64] = q^T  (queries are x rows 0:M)
            for ss in range(QS):
                pq = pp_pool.tile([64, 512], F32, tag="p")
                for c in range(DC):
                    nc.tensor.matmul(
                        pq[:],
                        wT[:, 0, c, :],
                        xT[:, c, ss * 512 : (ss + 1) * 512],
                        start=(c == 0),
                        stop=(c == DC - 1),
                    )
                nc.any.tensor_copy(qT_aug[0:64, ss * 512 : (ss + 1) * 512], pq[:])

            # squared norms via ones-matmul over partitions.
            # Engine writes may only start at partitions {0,32,64}, so:
            #  - kT_aug: memset rows 64:66 to 1, then overwrite row 64 with k2
            #  - qT_aug: matmul emits q2 into BOTH rows of a [2,512] psum,
            #    copy the pair to rows 64:66, then overwrite row 64 with 1
            tmp_sq = xp.tile([64, S], BF16)
            nc.vector.tensor_mul(tmp_sq[:, :], kT_aug[0:64, :], kT_aug[0:64, :])
            nc.vector.memset(kT_aug[64:66, :], 1.0)
            for ss in range(S // 512):
                p2 = pp_pool.tile([1, 512], F32, tag="p")
                nc.tensor.matmul(
                    p2[:],
                    ones64[:],
                    tmp_sq[:, ss * 512 : (ss + 1) * 512],
                    start=True,
                    stop=True,
                )
                # rows held -2k so the sum is 4*k2
                nc.vector.tensor_scalar_mul(
                    kT_aug[64:65, ss * 512 : (ss + 1) * 512], p2[:], 0.25
                )

            nc.vector.tensor_mul(tmp_sq[:, 0:M], qT_aug[0:64, :], qT_aug[0:64, :])
            for ss in range(QS):
                p2q = pp_pool.tile([2, 512], F32, tag="p")
                nc.tensor.matmul(
                    p2q[:],
                    ones64x2[:],
                    tmp_sq[:, ss * 512 : (ss + 1) * 512],
                    start=True,
                    stop=True,
                )
                nc.any.tensor_copy(qT_aug[64:66, ss * 512 : (ss + 1) * 512], p2q[:])
            nc.vector.memset(qT_aug[64:65, :], 1.0)

            # v projection: v_sb[:, n, 0:64] = x[128n:128n+128] @ Wv^T
            for n in range(KT):
                pv = pp_pool.tile([128, 64], F32, tag="p")
                for c in range(DC):
                    nc.tensor.matmul(
                        pv[:],
                        xT[:, c, n * 128 : (n + 1) * 128],
                        wT[:, 2, c, :],
                        start=(c == 0),
                        stop=(c == DC - 1),
                    )
                nc.any.tensor_copy(v_sb[:, n, 0:64], pv[:])
                nc.vector.memset(v_sb[:, n, 64:65], 1.0)

        # dist buffer reuses the SBUF freed by the setup pool
        dist_pool = ctx.enter_context(tc.tile_pool(name="dist", bufs=1))
        dist = dist_pool.tile([128, KT, M], F16)

        # ---------------- phase 1: d2 matmul + Sqrt -> dist ----------------
        with tc.tile_pool(name="ps", bufs=2, space="PSUM") as ps_pool:
            for n in range(KT):
                ps = ps_pool.tile([128, M], F32)
                for ss in range(QS):
                    nc.tensor.matmul(
                        ps[:, ss * 512 : (ss + 1) * 512],
                        kT_aug[:, n * 128 : (n + 1) * 128],
                        qT_aug[:, ss * 512 : (ss + 1) * 512],
                        start=True,
                        stop=True,
                    )
                # dist/8 = sqrt(d2/64)
                nc.scalar.activation(dist[:, n, :], ps[:], AF.Sqrt, scale=1.0 / 64.0)

        tc.no_sync_barrier()  # keep every Sqrt before every Exp (one table switch)

        # ---------------- phase 2: Exp + (att^T)@[v,1] ----------------
        with ExitStack() as p2ctx:
            po_pool = p2ctx.enter_context(
                tc.tile_pool(name="po", bufs=1, space="PSUM")
            )
            att_pool = p2ctx.enter_context(tc.tile_pool(name="att", bufs=3))
            po = po_pool.tile([65, M], F32)
            for n in range(KT):
                att = att_pool.tile([128, M], BF16)
                nc.scalar.activation(att[:], dist[:, n, :], AF.Exp)
                for ss in range(QS):
                    nc.tensor.matmul(
                        po[:, ss * 512 : (ss + 1) * 512],
                        v_sb[:, n, 0:65],
                        att[:, ss * 512 : (ss + 1) * 512],
                        start=(n == 0),
                        stop=(n == KT - 1),
                    )

            # -------- finish: transpose outT, normalize, store --------
            fin_pool = p2ctx.enter_context(tc.tile_pool(name="fin", bufs=2))
            oT_pool = p2ctx.enter_context(tc.tile_pool(name="oT", bufs=1))
            oT = oT_pool.tile([65, M], F32)
            nc.vector.tensor_copy(oT[:], po[:])
            pt_pool = p2ctx.enter_context(
                tc.tile_pool(name="pt", bufs=2, space="PSUM")
            )
            for j in range(M // 128):
                ptr = pt_pool.tile([128, 65], F32)
                nc.tensor.transpose(
                    ptr[:], oT[:, j * 128 : (j + 1) * 128], ident[0:65, 0:65]
                )
                r = fin_pool.tile([128, 1], F32, tag="recip")
                nc.vector.reciprocal(r[:], ptr[:, 64:65])
                ob = fin_pool.tile([128, DOUT], F32, tag="ob")
                nc.vector.tensor_scalar_mul(ob[:], ptr[:, 0:64], r[:])
                nc.sync.dma_start(out[j * 128 : (j + 1) * 128, :], ob[:])


_NC_CACHE = None


def build():
    global _NC_CACHE
    if _NC_CACHE is not None:
        return _NC_CACHE
    nc = bacc.Bacc("TRN2", target_bir_lowering=False, debug=False, num_devices=NCORES)
    x_d = nc.declare_dram_parameter("x", [S, DIN], F32, isOutput=False)
    wq_d = nc.declare_dram_parameter("wq", [DOUT, DIN], F32, isOutput=False)
    wk_d = nc.declare_dram_parameter("wk", [DOUT, DIN], F32, isOutput=False)
    wv_d = nc.declare_dram_parameter("wv", [DOUT, DIN], F32, isOutput=False)
    out_d = nc.declare_dram_parameter("out", [M, DOUT], F32, isOutput=True)
    with tile.TileContext(nc) as tc:
        _body(tc, x_d[:], wq_d[:], wk_d[:], wv_d[:], out_d[:])
    nc.compile()
    _NC_CACHE = nc
    return nc


def make_in_maps(x, Wq, Wk, Wv):
    wq = np.ascontiguousarray(Wq, np.float32)
    wk = np.ascontiguousarray(Wk, np.float32)
    wv = np.ascontiguousarray(Wv, np.float32)
    in_maps = []
    for c in range(NCORES):
        b, h = divmod(c, 2)
        xb = np.asarray(x[b], np.float32)
        xc = np.ascontiguousarray(
            np.concatenate([xb[h * M : (h + 1) * M], xb[(1 - h) * M : (2 - h) * M]], 0)
        )
        in_maps.append({"x": xc, "wq": wq, "wk": wk, "wv": wv})
    return in_maps


def gather_out(results):
    out = np.zeros((B, S, DOUT), np.float32)
    for c in range(NCORES):
        b, h = divmod(c, 2)
        out[b, h * M : (h + 1) * M] = results[c]["out"]
    return out


def kernel(x, Wq, Wk, Wv):
    nc = build()
    in_maps = make_in_maps(x, Wq, Wk, Wv)
    res = run_bass_kernel_spmd(nc, in_maps, core_ids=list(range(NCORES)))
    return gather_out(res.results)


# revision 10
# speedup vs baseline: 1.5564x; 1.5564x over previous
"""Trainium2 Bass kernel for AttentionL2 (B=4, S=4096, DIN=384, DOUT=64).

out = softmax(cdist(q, k) / 8, axis=-1) @ v  with q/k/v = x @ W{q,k,v}.T

Sharding: 8 cores = 4 batches x 2 query-halves. Each core receives the
full x of its batch (host pre-transposed to x^T for DMA layout) with
rows reordered so its own query half comes first (softmax over keys is
permutation invariant). Every core runs the same SPMD program:
q rows = columns 0:2048 of x^T, keys = all 4096.

Per-core math (matmuls bf16 with fp32 accumulation):
  d2[j,i] = |q_i - k_j|^2 via one augmented matmul:
      lhsT = [-2*k^T; k2; ones]  (66 x 128 keys per tile)
      rhs  = [q^T; ones; q2]     (66 x 2048)
  att = exp(sqrt(d2)/8), unnormalized (distances are O(10), no overflow,
  softmax needs no max subtraction), via two engine paths tile-by-tile:
   - ScalarE path: Sqrt(d2/64) -> fp16 buffer, then (after a scheduling
     barrier so the two ACT table sets load only once) Exp -> bf16
   - VectorE path: one custom DVE op p(d2)^2 with p a minimax cubic of
     exp(sqrt(z)/16) -- whole exp(sqrt(z)/8) in a single pass
  outT = [v; 1]^T @ att  (row 64 = softmax denominator, PSUM f32)
Final normalize outT[0:64]/outT[64] + transpose happen on the host.
"""

from contextlib import ExitStack

import numpy as np

import concourse.bacc as bacc
import concourse.mybir as mybir
import concourse.tile as tile
from concourse import dve_ops
from concourse.dve_spec import Spec, Src0, C0, C1, C2, One, lower
from concourse.dve_uop import DveOpSpec
from concourse.bass_utils import run_bass_kernel_spmd

F32 = mybir.dt.float32
BF16 = mybir.dt.bfloat16
F16 = mybir.dt.float16
AF = mybir.ActivationFunctionType

B, S, DIN, DOUT = 4, 4096, 384, 64
M = S // 2        # query rows per core
KT = S // 128     # 32 key tiles
DC = DIN // 128   # 3 contraction chunks
NCORES = 8
MMN = 512         # matmul moving free dim (psum out must stay in one bank)

# minimax cubic p for exp(sqrt(z)/16) on z in [32, 312], normalized by its
# constant term so the Horner tail can use the hardware One constant
# (a [P,1]-broadcast Src1 crashes the DVE, so only 3 scalar slots exist).
# att_dve = (p(z)/c0)^2 = exp(sqrt(z)/8)/c0^2; the ACT path matches the
# 1/c0^2 scale via a constant bias in its Exp (softmax is scale-invariant).
PA = 1.6518381642404523e-08
PB = -1.037933864407201e-05
PC = 0.006602996452846391
EXP_BIAS = -0.3424032850267295  # -2*ln(c0)

# key tiles handled by the custom-DVE composite path (rest: ACT sqrt/exp)
N_DVE = 16


def _register_dve_op():
    name = "EXP_SQRT_SQ_ANT"
    if name in dve_ops._SUB_OPCODE_FOR_NAME:
        return next(op for op in dve_ops.OPS if op.name == name)
    t = ((Src0 * C0 + C1) * Src0 + C2) * Src0 + One
    body = t * t

    def ref(in0, in1, c0, c1, c2):
        tt = ((in0 * c0 + c1) * in0 + c2) * in0 + 1.0
        return tt * tt

    spec = Spec(body=body, reference=ref)
    row = max(dve_ops._SUB_OPCODE_FOR_NAME.values()) + 1
    assert row < 0x20
    dve_ops._SUB_OPCODE_FOR_NAME[name] = row
    shas = {}
    for ver in ("v3", "v4"):
        try:
            uops = lower(spec, ver=ver)
            shas[ver] = DveOpSpec(
                name=name, opcode=row, uops=uops, rd1_en=False
            ).sha(ver)
        except Exception:
            pass
    op = dve_ops.DveOp(name, spec, subdim=False, uops_sha=shas)
    dve_ops.OPS.append(op)
    dve_ops.CUSTOM_DVE_SPECS[name] = spec
    return op


EXP_OP = _register_dve_op()


def _is_dve_tile(n):
    # spread DVE tiles evenly among the 32 key tiles
    return (n * N_DVE) % KT < N_DVE and n * N_DVE // KT < N_DVE


def _body(tc, xt, wt, out):
    nc = tc.nc
    dve_tiles = [n for n in range(KT) if _is_dve_tile(n)]
    assert len(dve_tiles) == N_DVE

    with ExitStack() as ctx:
        const_pool = ctx.enter_context(tc.tile_pool(name="const", bufs=1))
        ones64 = const_pool.tile([64, 1], BF16)
        nc.vector.memset(ones64[:], 1.0)
        ones64x2 = const_pool.tile([64, 2], BF16)
        nc.vector.memset(ones64x2[:], 1.0)
        ebias = const_pool.tile([128, 1], F32)
        nc.vector.memset(ebias[:], EXP_BIAS)
        main_pool = ctx.enter_context(tc.tile_pool(name="main", bufs=1))
        kT_aug = main_pool.tile([66, S], BF16)
        qT_aug = main_pool.tile([66, M], BF16)
        v_sb = main_pool.tile([128, KT, 65], BF16)

        # ---------------- setup: load x^T/W^T, cast, project ----------------
        with ExitStack() as sctx:
            xp = sctx.enter_context(tc.tile_pool(name="xsb", bufs=1))
            xT = xp.tile([128, DC, S], BF16)
            xTf = xp.tile([128, DC, S], F32, tag="xTf")
            wTf = xp.tile([128, DC, 3 * DOUT], F32)
            wT = xp.tile([128, DC, 3 * DOUT], BF16)
            xt_r = xt.rearrange("(c p) s -> p c s", p=128)
            wt_r = wt.rearrange("(c p) w -> p c w", p=128)
            for c in range(DC):
                nc.sync.dma_start(wTf[:, c, :], wt_r[:, c, :])
                nc.vector.tensor_copy(wT[:, c, :], wTf[:, c, :])
                for g in range(2):
                    sl = slice(g * (S // 2), (g + 1) * (S // 2))
                    nc.sync.dma_start(xTf[:, c, sl], xt_r[:, c, sl])
                    nc.vector.tensor_copy(xT[:, c, sl], xTf[:, c, sl])

            pp_pool = sctx.enter_context(
                tc.tile_pool(name="pp", bufs=3, space="PSUM")
            )
            # k rows: kT_aug[0:64] = -2 * k^T
            for ss in range(S // 512):
                pk = pp_pool.tile([64, 512], F32, tag="p")
                for c in range(DC):
                    nc.tensor.matmul(
                        pk[:],
                        wT[:, c, 64:128],
                        xT[:, c, ss * 512 : (ss + 1) * 512],
                        start=(c == 0),
                        stop=(c == DC - 1),
                    )
                nc.vector.tensor_scalar_mul(
                    kT_aug[0:64, ss * 512 : (ss + 1) * 512], pk[:], -2.0
                )
            # q rows: qT_aug[0:64] = q^T  (queries are x^T columns 0:M)
            for ss in range(M // 512):
                pq = pp_pool.tile([64, 512], F32, tag="p")
                for c in range(DC):
                    nc.tensor.matmul(
                        pq[:],
                        wT[:, c, 0:64],
                        xT[:, c, ss * 512 : (ss + 1) * 512],
                        start=(c == 0),
                        stop=(c == DC - 1),
                    )
                nc.vector.tensor_copy(qT_aug[0:64, ss * 512 : (ss + 1) * 512], pq[:])

            # squared norms via ones-matmul over partitions.
            # Engine writes may only start at partitions {0,32,64}:
            #  - kT_aug: memset rows 64:66 to 1, then overwrite row 64 with k2
            #  - qT_aug: emit q2 into both rows of a [2,512] psum, copy the
            #    pair to rows 64:66, then overwrite row 64 with 1
            tmp_sq = xp.tile([64, S], BF16, tag="sq")
            nc.vector.tensor_mul(tmp_sq[:, :], kT_aug[0:64, :], kT_aug[0:64, :])
            nc.vector.memset(kT_aug[64:66, :], 1.0)
            for ss in range(S // 512):
                p2 = pp_pool.tile([1, 512], F32, tag="p")
                nc.tensor.matmul(
                    p2[:],
                    ones64[:],
                    tmp_sq[:, ss * 512 : (ss + 1) * 512],
                    start=True,
                    stop=True,
                )
                # rows held -2k so the sum is 4*k2
                nc.vector.tensor_scalar_mul(
                    kT_aug[64:65, ss * 512 : (ss + 1) * 512], p2[:], 0.25
                )

            nc.vector.tensor_mul(tmp_sq[:, 0:M], qT_aug[0:64, :], qT_aug[0:64, :])
            for ss in range(M // 512):
                p2q = pp_pool.tile([2, 512], F32, tag="p")
                nc.tensor.matmul(
                    p2q[:],
                    ones64x2[:],
                    tmp_sq[:, ss * 512 : (ss + 1) * 512],
                    start=True,
                    stop=True,
                )
                nc.vector.tensor_copy(
                    qT_aug[64:66, ss * 512 : (ss + 1) * 512], p2q[:]
                )
            nc.vector.memset(qT_aug[64:65, :], 1.0)

            # v projection: v_sb[:, n, 0:64] = x[128n:128n+128] @ Wv^T
            for n in range(KT):
                pv = pp_pool.tile([128, 64], F32, tag="p")
                for c in range(DC):
                    nc.tensor.matmul(
                        pv[:],
                        xT[:, c, n * 128 : (n + 1) * 128],
                        wT[:, c, 128:192],
                        start=(c == 0),
                        stop=(c == DC - 1),
                    )
                nc.vector.tensor_copy(v_sb[:, n, 0:64], pv[:])
                nc.gpsimd.memset(v_sb[:, n, 64:65], 1.0)

        # shared buffer: fp16 dist (ACT tiles) or bf16 att (DVE tiles)
        buf_pool = ctx.enter_context(tc.tile_pool(name="buf", bufs=1))
        buf = buf_pool.tile([128, KT, M], F16)

        # ---------------- phase 1: d2 matmul + sqrt/composite ----------------
        with tc.tile_pool(name="ps", bufs=2, space="PSUM") as ps_pool:
            for n in range(KT):
                ps = ps_pool.tile([128, M], F32)
                for ss in range(M // MMN):
                    nc.tensor.matmul(
                        ps[:, ss * MMN : (ss + 1) * MMN],
                        kT_aug[:, n * 128 : (n + 1) * 128],
                        qT_aug[:, ss * MMN : (ss + 1) * MMN],
                        start=True,
                        stop=True,
                    )
                if _is_dve_tile(n):
                    # att = p(d2)^2 in one pass, written as bf16
                    nc.vector._custom_dve(
                        EXP_OP,
                        out=buf[:, n, :].bitcast(BF16),
                        in0=ps[:],
                        s0=PA,
                        s1=PB,
                        imm2=PC,
                    )
                else:
                    # dist/8 = sqrt(d2/64), fp16
                    nc.scalar.activation(
                        buf[:, n, :], ps[:], AF.Sqrt, scale=1.0 / 64.0
                    )

        tc.no_sync_barrier()  # all Sqrt before all Exp: one table switch

        # ---------------- phase 2: exp (ACT tiles) + [v;1]^T @ att ----------------
        with ExitStack() as p2ctx:
            po_pool = p2ctx.enter_context(
                tc.tile_pool(name="po", bufs=1, space="PSUM")
            )
            att_pool = p2ctx.enter_context(tc.tile_pool(name="att", bufs=3))
            po = po_pool.tile([65, M], F32)
            first, last = 0, KT - 1
            for n in range(KT):
                if _is_dve_tile(n):
                    att_ap = buf[:, n, :].bitcast(BF16)
                else:
                    att = att_pool.tile([128, M], BF16)
                    nc.scalar.activation(att[:], buf[:, n, :], AF.Exp, bias=ebias[:])
                    att_ap = att[:]
                for ss in range(M // MMN):
                    nc.tensor.matmul(
                        po[:, ss * MMN : (ss + 1) * MMN],
                        v_sb[:, n, 0:65],
                        att_ap[:, ss * MMN : (ss + 1) * MMN],
                        start=(n == first),
                        stop=(n == last),
                    )

            # -------- finish: copy outT to SBUF, DMA out --------
            oT_pool = p2ctx.enter_context(tc.tile_pool(name="oT", bufs=1))
            oT = oT_pool.tile([65, M], F32)
            nc.vector.tensor_copy(oT[:], po[:])
            nc.sync.dma_start(out[:, :], oT[:])


_NC_CACHE = None


def build():
    global _NC_CACHE
    if _NC_CACHE is not None:
        return _NC_CACHE
    nc = bacc.Bacc("TRN2", target_bir_lowering=False, debug=False, num_devices=NCORES)
    xt_d = nc.declare_dram_parameter("xt", [DIN, S], F32, isOutput=False)
    wt_d = nc.declare_dram_parameter("wt", [DIN, 3 * DOUT], F32, isOutput=False)
    out_d = nc.declare_dram_parameter("out", [65, M], F32, isOutput=True)
    with tile.TileContext(nc) as tc:
        _body(tc, xt_d[:], wt_d[:], out_d[:])
    nc.compile()
    _NC_CACHE = nc
    return nc


def make_in_maps(x, Wq, Wk, Wv):
    wt = np.ascontiguousarray(
        np.concatenate(
            [np.asarray(W, np.float32).T for W in (Wq, Wk, Wv)], axis=1
        )
    )
    in_maps = []
    for c in range(NCORES):
        b, h = divmod(c, 2)
        xb = np.asarray(x[b], np.float32)
        xc = np.concatenate(
            [xb[h * M : (h + 1) * M], xb[(1 - h) * M : (2 - h) * M]], 0
        )
        in_maps.append({"xt": np.ascontiguousarray(xc.T), "wt": wt})
    return in_maps


def gather_out(results):
    out = np.zeros((B, S, DOUT), np.float32)
    for c in range(NCORES):
        b, h = divmod(c, 2)
        oT = results[c]["out"]
        out[b, h * M : (h + 1) * M] = (oT[0:64] / oT[64:65]).T
    return out


def kernel(x, Wq, Wk, Wv):
    nc = build()
    in_maps = make_in_maps(x, Wq, Wk, Wv)
    res = run_bass_kernel_spmd(nc, in_maps, core_ids=list(range(NCORES)))
    return gather_out(res.results)


# revision 11
# speedup vs baseline: 1.5891x; 1.0210x over previous
"""Trainium2 Bass kernel for AttentionL2 (B=4, S=4096, DIN=384, DOUT=64).

out = softmax(cdist(q, k) / 8, axis=-1) @ v  with q/k/v = x @ W{q,k,v}.T

Sharding: 8 cores = 4 batches x 2 query-halves. Each core receives the
full x of its batch, host pre-transposed to x^T (bf16 -- identical to
the on-chip cast the matmul needs anyway) with rows reordered so its own
query half comes first (softmax over keys is permutation invariant).
Every core runs the same SPMD program: q = columns 0:2048, keys = all.

Per-core math (matmuls bf16 with fp32 accumulation):
  d2[j,i] = |q_i - k_j|^2 via one augmented matmul with the contraction
  padded to 128 rows (zeros) so the PE's activity monitor unthrottles:
      lhsT = [-2*k^T; k2; 1; 0...]  (128 x 128 keys per tile)
      rhs  = [q^T; 1; q2; 0...]     (128 x 2048)
  att = exp(sqrt(d2)/8) (unnormalized; distances are O(10), no overflow,
  softmax needs no max subtraction), two engine paths tile-by-tile:
   - ScalarE: Sqrt(d2/64) -> fp16 buffer; after a scheduler barrier (one
     ACT table switch) Exp with bias -2*ln(c0) -> bf16
   - VectorE: one custom DVE op (p(z)/c0)^2, p = minimax cubic of
     exp(sqrt(z)/16): the whole exp(sqrt(z)/8)/c0^2 in a single pass
  outT = [v; 1; 0...]^T @ att  (row 64 = softmax denominator, PSUM f32)
Final normalize outT[0:64]/outT[64] + transpose happen on the host.
"""

from contextlib import ExitStack

import ml_dtypes
import numpy as np

import concourse.bacc as bacc
import concourse.mybir as mybir
import concourse.tile as tile
from concourse import dve_ops
from concourse.dve_spec import Spec, Src0, C0, C1, C2, One, lower
from concourse.dve_uop import DveOpSpec
from concourse.bass_utils import run_bass_kernel_spmd

F32 = mybir.dt.float32
BF16 = mybir.dt.bfloat16
F16 = mybir.dt.float16
AF = mybir.ActivationFunctionType

B, S, DIN, DOUT = 4, 4096, 384, 64
M = S // 2        # query rows per core
KT = S // 128     # 32 key tiles
DC = DIN // 128   # 3 contraction chunks
NCORES = 8
MMN = 512         # matmul moving free dim (psum out must stay in one bank)

# minimax cubic p for exp(sqrt(z)/16) on z in [32, 312], normalized by its
# constant term so the Horner tail can use the hardware One constant
# (a [P,1]-broadcast Src1 crashes the DVE, so only 3 scalar slots exist).
# att_dve = (p(z)/c0)^2 = exp(sqrt(z)/8)/c0^2; the ACT path matches the
# 1/c0^2 scale via a constant bias in its Exp (softmax is scale-invariant).
PA = 1.6518381642404523e-08
PB = -1.037933864407201e-05
PC = 0.006602996452846391
EXP_BIAS = -0.3424032850267295  # -2*ln(c0)

# key tiles handled by the custom-DVE composite path (rest: ACT sqrt/exp)
N_DVE = 15


def _register_dve_op():
    name = "EXP_SQRT_SQ_ANT"
    if name in dve_ops._SUB_OPCODE_FOR_NAME:
        return next(op for op in dve_ops.OPS if op.name == name)
    t = ((Src0 * C0 + C1) * Src0 + C2) * Src0 + One
    body = t * t

    def ref(in0, in1, c0, c1, c2):
        tt = ((in0 * c0 + c1) * in0 + c2) * in0 + 1.0
        return tt * tt

    spec = Spec(body=body, reference=ref)
    row = max(dve_ops._SUB_OPCODE_FOR_NAME.values()) + 1
    assert row < 0x20
    dve_ops._SUB_OPCODE_FOR_NAME[name] = row
    shas = {}
    for ver in ("v3", "v4"):
        try:
            uops = lower(spec, ver=ver)
            shas[ver] = DveOpSpec(
                name=name, opcode=row, uops=uops, rd1_en=False
            ).sha(ver)
        except Exception:
            pass
    op = dve_ops.DveOp(name, spec, subdim=False, uops_sha=shas)
    dve_ops.OPS.append(op)
    dve_ops.CUSTOM_DVE_SPECS[name] = spec
    return op


EXP_OP = _register_dve_op()


def _is_dve_tile(n):
    # spread DVE tiles evenly among the 32 key tiles
    return (n * N_DVE) % KT >= KT - N_DVE


def _body(tc, xt, wt, out):
    nc = tc.nc
    assert sum(_is_dve_tile(n) for n in range(KT)) == N_DVE

    with ExitStack() as ctx:
        const_pool = ctx.enter_context(tc.tile_pool(name="const", bufs=1))
        ones64 = const_pool.tile([64, 1], BF16)
        nc.vector.memset(ones64[:], 1.0)
        ones64x2 = const_pool.tile([64, 2], BF16)
        nc.vector.memset(ones64x2[:], 1.0)
        ebias = const_pool.tile([128, 1], F32)
        nc.vector.memset(ebias[:], EXP_BIAS)

        main_pool = ctx.enter_context(tc.tile_pool(name="main", bufs=1))
        kT_aug = main_pool.tile([128, S], BF16)
        qT_aug = main_pool.tile([128, M], BF16)
        v_sb = main_pool.tile([128, KT, 128], BF16)

        # ---------------- setup: load x^T/W^T (bf16), project ----------------
        with ExitStack() as sctx:
            xp = sctx.enter_context(tc.tile_pool(name="xsb", bufs=1))
            xT = xp.tile([128, DC, S], BF16)
            wT = xp.tile([128, DC, 3 * DOUT], BF16)
            vT = xp.tile([64, S], BF16)
            xt_r = xt.rearrange("(c p) s -> p c s", p=128)
            wt_r = wt.rearrange("(c p) w -> p c w", p=128)
            for c in range(DC):
                nc.sync.dma_start(wT[:, c, :], wt_r[:, c, :])
                for g in range(2):
                    sl = slice(g * (S // 2), (g + 1) * (S // 2))
                    nc.sync.dma_start(xT[:, c, sl], xt_r[:, c, sl])

            pp_pool = sctx.enter_context(
                tc.tile_pool(name="pp", bufs=3, space="PSUM")
            )
            # k rows: kT_aug[0:64] = -2 * k^T ; v^T rows for the transpose
            for ss in range(S // 512):
                pk = pp_pool.tile([64, 512], F32, tag="p")
                for c in range(DC):
                    nc.tensor.matmul(
                        pk[:],
                        wT[:, c, 64:128],
                        xT[:, c, ss * 512 : (ss + 1) * 512],
                        start=(c == 0),
                        stop=(c == DC - 1),
                    )
                nc.vector.tensor_scalar_mul(
                    kT_aug[0:64, ss * 512 : (ss + 1) * 512], pk[:], -2.0
                )
                pv = pp_pool.tile([64, 512], F32, tag="p")
                for c in range(DC):
                    nc.tensor.matmul(
                        pv[:],
                        wT[:, c, 128:192],
                        xT[:, c, ss * 512 : (ss + 1) * 512],
                        start=(c == 0),
                        stop=(c == DC - 1),
                    )
                nc.vector.tensor_copy(vT[:, ss * 512 : (ss + 1) * 512], pv[:])
            # q rows: qT_aug[0:64] = q^T  (queries are x^T columns 0:M)
            for ss in range(M // 512):
                pq = pp_pool.tile([64, 512], F32, tag="p")
                for c in range(DC):
                    nc.tensor.matmul(
                        pq[:],
                        wT[:, c, 0:64],
                        xT[:, c, ss * 512 : (ss + 1) * 512],
                        start=(c == 0),
                        stop=(c == DC - 1),
                    )
                nc.vector.tensor_copy(qT_aug[0:64, ss * 512 : (ss + 1) * 512], pq[:])

            # v_sb[:, n, j] = v[128n+p, j] via SBUF->SBUF xbar DMA transpose;
            # column 64 = 1 (softmax denominator), columns 65:128 = 0
            nc.vector.memset(v_sb[:, :, 64:128], 0.0)
            for n in range(KT):
                nc.sync.dma_start_transpose(
                    v_sb[:, n, 0:64], vT[:, n * 128 : (n + 1) * 128]
                )
            nc.gpsimd.memset(v_sb[:, :, 64:65], 1.0)

            # squared norms via ones-matmul over partitions. Engine writes
            # may only start at partitions {0,32,64}, hence the ordering:
            #  - kT_aug: rows[64:128]=0, rows[64:66]=1, then row 64 <- k2
            #  - qT_aug: rows[64:128]=0, q2 pair -> rows[64:66], row 64 <- 1
            tmp_sq = xp.tile([64, S], BF16, tag="sq")
            nc.vector.tensor_mul(tmp_sq[:, :], kT_aug[0:64, :], kT_aug[0:64, :])
            nc.vector.memset(kT_aug[64:128, :], 0.0)
            nc.vector.memset(kT_aug[64:66, :], 1.0)
            for ss in range(S // 512):
                p2 = pp_pool.tile([1, 512], F32, tag="p")
                nc.tensor.matmul(
                    p2[:],
                    ones64[:],
                    tmp_sq[:, ss * 512 : (ss + 1) * 512],
                    start=True,
                    stop=True,
                )
                # rows held -2k so the sum is 4*k2
                nc.vector.tensor_scalar_mul(
                    kT_aug[64:65, ss * 512 : (ss + 1) * 512], p2[:], 0.25
                )

            nc.vector.tensor_mul(tmp_sq[:, 0:M], qT_aug[0:64, :], qT_aug[0:64, :])
            nc.vector.memset(qT_aug[64:128, :], 0.0)
            for ss in range(M // 512):
                p2q = pp_pool.tile([2, 512], F32, tag="p")
                nc.tensor.matmul(
                    p2q[:],
                    ones64x2[:],
                    tmp_sq[:, ss * 512 : (ss + 1) * 512],
                    start=True,
                    stop=True,
                )
                nc.vector.tensor_copy(
                    qT_aug[64:66, ss * 512 : (ss + 1) * 512], p2q[:]
                )
            nc.vector.memset(qT_aug[64:65, :], 1.0)

        # shared buffer: fp16 dist (ACT tiles) or bf16 att (DVE tiles)
        buf_pool = ctx.enter_context(tc.tile_pool(name="buf", bufs=1))
        buf = buf_pool.tile([128, KT, M], F16)

        # ---------------- phase 1: d2 matmul + sqrt/composite ----------------
        with tc.tile_pool(name="ps", bufs=2, space="PSUM") as ps_pool:
            for n in range(KT):
                ps = ps_pool.tile([128, M], F32)
                for ss in range(M // MMN):
                    nc.tensor.matmul(
                        ps[:, ss * MMN : (ss + 1) * MMN],
                        kT_aug[:, n * 128 : (n + 1) * 128],
                        qT_aug[:, ss * MMN : (ss + 1) * MMN],
                        start=True,
                        stop=True,
                    )
                if _is_dve_tile(n):
                    # att/c0^2 = (p(d2)/c0)^2 in one pass, written as bf16
                    nc.vector._custom_dve(
                        EXP_OP,
                        out=buf[:, n, :].bitcast(BF16),
                        in0=ps[:],
                        s0=PA,
                        s1=PB,
                        imm2=PC,
                    )
                else:
                    # dist/8 = sqrt(d2/64), fp16
                    nc.scalar.activation(
                        buf[:, n, :], ps[:], AF.Sqrt, scale=1.0 / 64.0
                    )

        tc.no_sync_barrier()  # all Sqrt before all Exp: one table switch

        # ---------------- phase 2: exp (ACT tiles) + [v;1]^T @ att ----------------
        with ExitStack() as p2ctx:
            po_pool = p2ctx.enter_context(
                tc.tile_pool(name="po", bufs=1, space="PSUM")
            )
            att_pool = p2ctx.enter_context(tc.tile_pool(name="att", bufs=3))
            po = po_pool.tile([128, M], F32)
            for n in range(KT):
                if _is_dve_tile(n):
                    att_ap = buf[:, n, :].bitcast(BF16)
                else:
                    att = att_pool.tile([128, M], BF16)
                    nc.scalar.activation(
                        att[:], buf[:, n, :], AF.Exp, bias=ebias[:]
                    )
                    att_ap = att[:]
                for ss in range(M // MMN):
                    nc.tensor.matmul(
                        po[:, ss * MMN : (ss + 1) * MMN],
                        v_sb[:, n, :],
                        att_ap[:, ss * MMN : (ss + 1) * MMN],
                        start=(n == 0),
                        stop=(n == KT - 1),
                    )

            # -------- finish: copy outT[0:65] to SBUF, DMA out --------
            oT_pool = p2ctx.enter_context(tc.tile_pool(name="oT", bufs=1))
            oT = oT_pool.tile([65, M], F32)
            nc.vector.tensor_copy(oT[:], po[0:65, :])
            nc.sync.dma_start(out[:, :], oT[:])


_NC_CACHE = None


def build():
    global _NC_CACHE
    if _NC_CACHE is not None:
        return _NC_CACHE
    nc = bacc.Bacc("TRN2", target_bir_lowering=False, debug=False, num_devices=NCORES)
    xt_d = nc.declare_dram_parameter("xt", [DIN, S], BF16, isOutput=False)
    wt_d = nc.declare_dram_parameter("wt", [DIN, 3 * DOUT], BF16, isOutput=False)
    out_d = nc.declare_dram_parameter("out", [65, M], F32, isOutput=True)
    with tile.TileContext(nc) as tc:
        _body(tc, xt_d[:], wt_d[:], out_d[:])
    nc.compile()
    _NC_CACHE = nc
    return nc


def make_in_maps(x, Wq, Wk, Wv):
    bf16 = ml_dtypes.bfloat16
    wt = np.ascontiguousarray(
        np.concatenate(
            [np.asarray(W, np.float32).T for W in (Wq, Wk, Wv)], axis=1
        ).astype(bf16)
    )
    in_maps = []
    for c in range(NCORES):
        b, h = divmod(c, 2)
        xb = np.asarray(x[b], np.float32)
        xc = np.concatenate(
            [xb[h * M : (h + 1) * M], xb[(1 - h) * M : (2 - h) * M]], 0
        )
        in_maps.append({"xt": np.ascontiguousarray(xc.T.astype(bf16)), "wt": wt})
    return in_maps


def gather_out(results):
    out = np.zeros((B, S, DOUT), np.float32)
    for c in range(NCORES):
        b, h = divmod(c, 2)
        oT = np.asarray(results[c]["out"], np.float32)
        out[b, h * M : (h + 1) * M] = (oT[0:64] / oT[64:65]).T
    return out


def kernel(x, Wq, Wk, Wv):
    nc = build()
    in_maps = make_in_maps(x, Wq, Wk, Wv)
    res = run_bass_kernel_spmd(nc, in_maps, core_ids=list(range(NCORES)))
    return gather_out(res.results)
